# revision 6
# baseline (speedup 1.0000x reference)
"""Trainium2 Bass kernel for char-CNN: 5-tap conv along word_length + max-pool.

Reference computation (per (batch, sentence) word, shapes B=64 S=256 W=20 E=128):
    y[w, e] = sum_{kh=0..4} x[w + kh - 2, e] * conv_w[kh]     (zero padded)
    out[e]  = max_w y[w, e] + conv_b

Strategy:
  - Data-parallel over 8 NeuronCores: 8 batches (2048 words) per core.
  - Host pre-arranges each core's shard to z[(j w)=120, group=342, e=128]
    (groups of J=6 words, last group zero-padded) and casts f32 -> f16, so
    HBM traffic is halved and every DMA descriptor is a multi-KiB
    contiguous run per partition.
  - The whole 10.5 MB input shard fits in SBUF (87.5 KiB/partition), so
    the entire input stream is issued up front as a handful of >=1 MB
    DMAs alternating across the two HWDGE rings (sync / scalar): RTL
    descriptor generation, no SWDGE software-emission bottleneck, and no
    compute op can head-of-line-block a DMA trigger.
  - The conv is a banded 20x20 matrix applied per word, done on TensorE:
    stationary lhsT = x6 [K=120 (6 words x 20 w_in), M=128 (e)], moving
    rhs = block-diagonal A [120, 120] -> PSUM [128 (e), 120 (6w x 20 w_out)].
    fp16 operands (1 cycle/row on PE; fp32 would be 4).
  - Max over w_out: the f32 PSUM drain is the scarce resource (ACT and
    DVE each move 1 elem/cycle/lane out of PSUM; tensor ops may read at
    most ONE operand from PSUM).  Per PSUM bank the work alternates:
      path A: ACT copies the whole bank f32->f16 to SBUF; DVE does the
              20->10 max level in f16 2x mode (fast).
      path B: ACT copies only w-planes 10..19 f32->f16; DVE does the
              20->10 level as max(PSUM[w<10] (f32), SBUF half (f16)), 1x.
    This splits the drain evenly (ACT ~= DVE ~= 75 ns/group).  Levels
    2-5 are contiguous step-1 f16 tensor_max (DVE 2x mode) landing in a
    persistent [128, NW] maxima tile, flushed to DRAM in large pieces on
    the sync ring.
"""

from contextlib import ExitStack

import numpy as np

import concourse.bass as bass
import concourse.mybir as mybir
import concourse.tile as tile
from concourse import bacc

W = 20  # word length
E = 128  # embedding dim
KH = 5  # conv taps
PAD = 2
J = 6  # words per matmul group (6 * 20 = 120 <= 128 partitions)
KP = J * W  # contraction size / partitions used (120)
NCORES = 8
BANK = 512  # PSUM bank size in f32 elements
HW = W // 2  # 10


def build_conv_matrix(conv_w: np.ndarray) -> np.ndarray:
    """[KP, KP] conv matrix, output columns ordered w_out-major:
    A[j*W + wi, wo*J + j] = conv_w[wi - wo + 2].  The w-major column
    order makes the PSUM output planar so every max level on VectorE is
    a contiguous step-1 tensor_max."""
    wv = np.asarray(conv_w, np.float32).reshape(-1)
    assert wv.shape == (KH,)
    a = np.zeros((KP, KP), np.float32)
    for j in range(J):
        for wo in range(W):
            for kh in range(KH):
                wi = wo + kh - PAD
                if 0 <= wi < W:
                    a[j * W + wi, wo * J + j] = wv[kh]
    return a.astype(np.float16)


def pack_input(x_core: np.ndarray, ng: int) -> np.ndarray:
    """[nw, W, E] f32 -> [KP, ng, E] f16 partition-major, zero-padded to
    ng*J words. The fp16 cast is the same one the kernel's compute path
    uses (TensorE consumes fp16); doing it host-side halves HBM traffic."""
    nw = x_core.shape[0]
    xp = np.zeros((ng * J, W, E), np.float16)
    xp[:nw] = x_core.astype(np.float16)
    # (g j) w e -> (j w) g e
    return np.ascontiguousarray(
        xp.reshape(ng, J, W, E).transpose(1, 2, 0, 3).reshape(KP, ng, E)
    )


def chunk_plan(ng: int) -> list[int]:
    """Small chunks first (compute starts early), 64-group (~1.9 MB) in
    the middle (big descriptors, few triggers), small tail (short
    pipeline drain after the last byte lands)."""
    sizes = []
    rem = ng
    for warm in (16, 32):
        if rem >= warm + 64:
            sizes.append(warm)
            rem -= warm
    while rem >= 72:
        sizes.append(64)
        rem -= 64
    if rem > 8:
        sizes.append(rem - 6)
        rem = 6
    if rem:
        sizes.append(rem)
    return sizes


def build_nc(
    nw: int,
    rings: tuple[str, ...] = ("sync", "scalar"),
    cg: int = 16,
    psum_bufs: int = 2,
    flush_words: int = 384,
    tree_bufs: int = 3,
    bank_pattern: str = "AB",
) -> bass.Bass:
    """Build the per-core Bass graph. nw = real words per core."""
    f32 = mybir.dt.float32
    f16 = mybir.dt.float16
    ng = (nw + J - 1) // J  # padded group count
    nwp = ng * J  # padded word count

    nc = bacc.Bacc()
    z_ext = nc.declare_dram_parameter("z", [KP, ng, E], f16, isOutput=False)
    a_ext = nc.declare_dram_parameter("a", [KP, KP], f16, isOutput=False)
    out_ext = nc.declare_dram_parameter("out", [E, nw], f16, isOutput=True)

    engines = {
        "sync": nc.sync,
        "scalar": nc.scalar,
        "gpsimd": nc.gpsimd,
    }

    with ExitStack() as ctx:
        tc = ctx.enter_context(tile.TileContext(nc))
        const = ctx.enter_context(tc.tile_pool(name="const", bufs=1))
        xin = ctx.enter_context(tc.tile_pool(name="xin", bufs=1))
        opool = ctx.enter_context(tc.tile_pool(name="o", bufs=1))
        spool = ctx.enter_context(tc.tile_pool(name="s", bufs=2 * tree_bufs))
        t1pool = ctx.enter_context(tc.tile_pool(name="t1", bufs=tree_bufs))
        u2pool = ctx.enter_context(tc.tile_pool(name="u2", bufs=tree_bufs))
        u3pool = ctx.enter_context(tc.tile_pool(name="u3", bufs=tree_bufs))
        u4pool = ctx.enter_context(tc.tile_pool(name="u4", bufs=tree_bufs))
        pspool = ctx.enter_context(
            tc.tile_pool(name="ps", bufs=psum_bufs, space="PSUM")
        )

        a_t = const.tile([KP, KP], f16)
        nc.sync.dma_start(out=a_t[:, :], in_=a_ext[:, :])
        maxt = opool.tile([E, nwp], f16)

        # Phase A: the entire input stream, issued up front into
        # persistent SBUF tiles, alternating across the HWDGE rings.
        sizes = chunk_plan(ng)
        chunks = []
        g0 = 0
        for ci, gn in enumerate(sizes):
            xt = xin.tile([KP, gn * E], f16, tag=f"x{ci}")
            src = z_ext[:, g0 : g0 + gn, :].rearrange("p g e -> p (g e)")
            engines[rings[ci % len(rings)]].dma_start(out=xt[:, :], in_=src)
            chunks.append((xt, g0, gn))
            g0 += gn

        # Phase B: compute pipeline, cg groups (= 4 PSUM banks) at a time.
        w_flushed = 0

        def flush_out(upto_words):
            nonlocal w_flushed
            hi = min(upto_words, nw)
            if hi - w_flushed >= flush_words or (hi >= nw and hi > w_flushed):
                nc.sync.dma_start(
                    out=out_ext[:, w_flushed:hi], in_=maxt[:, w_flushed:hi]
                )
                w_flushed = hi

        bank_idx = 0
        for xt, gc, gn in chunks:
            for s0 in range(0, gn, cg):
                sn = min(cg, gn - s0)
                sg0 = gc + s0
                nb = (sn + 3) // 4
                ps = pspool.tile([E, 4 * BANK], f32, tag="ps")
                for g in range(sn):
                    col = (g // 4) * BANK + (g % 4) * KP
                    nc.tensor.matmul(
                        ps[:, col : col + KP],
                        lhsT=xt[:, (s0 + g) * E : (s0 + g + 1) * E],
                        rhs=a_t[:, :],
                        start=True,
                        stop=True,
                    )
                # Level 1 (20 -> 10) per PSUM bank, path A or B.
                u1 = t1pool.tile([E, cg * J * HW], f16, tag="u1")
                for b in range(nb):
                    gb = min(4, sn - 4 * b)
                    pv = ps[:, b * BANK : b * BANK + gb * J * W].rearrange(
                        "p (g w j) -> p g w j", w=W, j=J
                    )
                    ov = u1[
                        :, b * 4 * J * HW : (b * 4 + gb) * J * HW
                    ].rearrange("p (g w j) -> p g w j", w=HW, j=J)
                    path = bank_pattern[bank_idx % len(bank_pattern)]
                    bank_idx += 1
                    if path == "A":
                        # ACT drains the whole bank; DVE maxes in f16 2x.
                        s = spool.tile([E, 4 * J * W], f16, tag="sa")
                        sv = s[:, 0 : gb * J * W].rearrange(
                            "p (g w j) -> p g w j", w=W, j=J
                        )
                        nc.scalar.copy(sv, pv)
                        nc.vector.tensor_max(
                            ov, sv[:, :, 0:HW, :], sv[:, :, HW:W, :]
                        )
                    else:
                        # ACT drains w-planes 10..19 only; DVE maxes the
                        # PSUM low half against the SBUF high half (1x).
                        sh = spool.tile([E, 4 * J * HW], f16, tag="sb")
                        shv = sh[:, 0 : gb * J * HW].rearrange(
                            "p (g w j) -> p g w j", w=HW, j=J
                        )
                        nc.scalar.copy(shv, pv[:, :, HW:W, :])
                        nc.vector.tensor_max(ov, pv[:, :, 0:HW, :], shv)

                # Levels 2-5: contiguous f16 2x-mode maxes in SBUF:
                # 10 -> 5 -> (2,2,1) -> 1.
                def v(t, nw_, w0, w1):
                    return t[:, 0 : sn * nw_ * J].rearrange(
                        "p (g w j) -> p g w j", g=sn, j=J
                    )[:, :, w0:w1, :]

                u2 = u2pool.tile([E, cg * J * 5], f16, tag="u2")
                nc.vector.tensor_max(
                    u2[:, 0 : sn * 5 * J], v(u1, HW, 0, 5), v(u1, HW, 5, 10)
                )
                u3 = u3pool.tile([E, cg * J * 2], f16, tag="u3")
                nc.vector.tensor_max(
                    u3[:, 0 : sn * 2 * J], v(u2, 5, 0, 2), v(u2, 5, 2, 4)
                )
                u4 = u4pool.tile([E, cg * J], f16, tag="u4")
                nc.vector.tensor_max(
                    u4[:, 0 : sn * J], v(u3, 2, 0, 1), v(u3, 2, 1, 2)
                )
                nc.vector.tensor_max(
                    maxt[:, sg0 * J : (sg0 + sn) * J].rearrange(
                        "p (g w j) -> p g w j", g=sn, j=J
                    ),
                    v(u4, 1, 0, 1),
                    v(u2, 5, 4, 5),
                )
                flush_out((sg0 + sn) * J)
    nc.finalize()
    return nc


def kernel(embedded_char, conv_w, conv_b):
    from concourse.bass_utils import run_bass_kernel_spmd

    x = np.asarray(embedded_char, np.float32)
    b_val = float(np.asarray(conv_b, np.float32).reshape(-1)[0])
    B, S, Wl, El = x.shape
    assert (Wl, El) == (W, E)
    bs = B // NCORES
    nw = bs * S
    ng = (nw + J - 1) // J
    a16 = build_conv_matrix(conv_w)

    nc = build_nc(nw)
    in_maps = [
        {
            "z": pack_input(x[i * bs : (i + 1) * bs].reshape(nw, Wl, El), ng),
            "a": a16,
        }
        for i in range(NCORES)
    ]
    res = run_bass_kernel_spmd(nc, in_maps, core_ids=list(range(NCORES)))
    full = np.concatenate(
        [r["out"].astype(np.float32).T.reshape(bs, S, El) for r in res.results], axis=0
    )
    if b_val != 0.0:
        full = full + b_val
    return np.ascontiguousarray(full.astype(np.float32))


# revision 9
# speedup vs baseline: 1.1998x; 1.1998x over previous
"""Trainium2 Bass kernel for char-CNN: 5-tap conv along word_length + max-pool.

Reference computation (per (batch, sentence) word, shapes B=64 S=256 W=20 E=128):
    y[w, e] = sum_{kh=0..4} x[w + kh - 2, e] * conv_w[kh]     (zero padded)
    out[e]  = max_w y[w, e] + conv_b

Strategy:
  - Data-parallel over 8 NeuronCores: 8 batches (2048 words) per core.
  - Host pre-arranges each core's shard to z[(j w)=120, group=342, e=128]
    (groups of J=6 words, last group zero-padded) and casts f32 -> f16, so
    HBM traffic is halved and every DMA descriptor is a multi-KiB
    contiguous run per partition.
  - The whole 10.5 MB input shard fits in SBUF (87.5 KiB/partition), so
    the entire input stream is issued up front as a handful of >=1 MB
    DMAs alternating across the two HWDGE rings (sync / scalar): RTL
    descriptor generation, no SWDGE software-emission bottleneck, and no
    compute op can head-of-line-block a DMA trigger.
  - The conv is a banded 20x20 matrix applied per word, done on TensorE:
    stationary lhsT = x6 [K=120 (6 words x 20 w_in), M=128 (e)], moving
    rhs = block-diagonal A [120, 120] -> PSUM [128 (e), 120 (6w x 20 w_out)].
    fp16 operands (1 cycle/row on PE; fp32 would be 4).
  - Max over w_out: the f32 PSUM drain is the scarce resource (ACT and
    DVE each move 1 elem/cycle/lane out of PSUM; tensor ops may read at
    most ONE operand from PSUM).  Per PSUM bank the work alternates:
      path A: ACT copies the whole bank f32->f16 to SBUF; DVE does the
              20->10 max level in f16 2x mode (fast).
      path B: ACT copies only w-planes 10..19 f32->f16; DVE does the
              20->10 level as max(PSUM[w<10] (f32), SBUF half (f16)), 1x.
    This splits the drain evenly (ACT ~= DVE ~= 75 ns/group).  Levels
    2-5 are contiguous step-1 f16 tensor_max (DVE 2x mode) landing in a
    persistent [128, NW] maxima tile, flushed to DRAM in large pieces on
    the sync ring.
"""

from contextlib import ExitStack

import numpy as np

import concourse.bass as bass
import concourse.mybir as mybir
import concourse.tile as tile
from concourse import bacc

W = 20  # word length
E = 128  # embedding dim
KH = 5  # conv taps
PAD = 2
J = 6  # words per matmul group (6 * 20 = 120 <= 128 partitions)
KP = J * W  # contraction size / partitions used (120)
NCORES = 8
BANK = 512  # PSUM bank size in f32 elements
HW = W // 2  # 10


def build_conv_matrix(conv_w: np.ndarray) -> np.ndarray:
    """[KP, KP] conv matrix, output columns ordered w_out-major:
    A[j*W + wi, wo*J + j] = conv_w[wi - wo + 2].  The w-major column
    order makes the PSUM output planar so every max level on VectorE is
    a contiguous step-1 tensor_max."""
    wv = np.asarray(conv_w, np.float32).reshape(-1)
    assert wv.shape == (KH,)
    a = np.zeros((KP, KP), np.float32)
    for j in range(J):
        for wo in range(W):
            for kh in range(KH):
                wi = wo + kh - PAD
                if 0 <= wi < W:
                    a[j * W + wi, wo * J + j] = wv[kh]
    return a.astype(np.float16)


def pack_input(x_core: np.ndarray, ng: int) -> np.ndarray:
    """[nw, W, E] f32 -> [KP, ng, E] f16 partition-major, zero-padded to
    ng*J words. The fp16 cast is the same one the kernel's compute path
    uses (TensorE consumes fp16); doing it host-side halves HBM traffic."""
    nw = x_core.shape[0]
    xp = np.zeros((ng * J, W, E), np.float16)
    xp[:nw] = x_core.astype(np.float16)
    # (g j) w e -> (j w) g e
    return np.ascontiguousarray(
        xp.reshape(ng, J, W, E).transpose(1, 2, 0, 3).reshape(KP, ng, E)
    )


def chunk_plan(ng: int) -> list[int]:
    """Small chunks first (compute starts early), 64-group (~1.9 MB) in
    the middle (big descriptors, few triggers), small tail (short
    pipeline drain after the last byte lands)."""
    sizes = []
    rem = ng
    for warm in (8, 24):
        if rem >= warm + 64:
            sizes.append(warm)
            rem -= warm
    while rem >= 72:
        sizes.append(64)
        rem -= 64
    if rem > 8:
        sizes.append(rem - 6)
        rem = 6
    if rem:
        sizes.append(rem)
    return sizes


def build_nc(
    nw: int,
    rings: tuple[str, ...] = ("sync",),
    out_ring: str = "gpsimd",
    cg: int = 16,
    psum_bufs: int = 2,
    flush_words: int = 384,
    tree_bufs: int = 3,
    bank_pattern: str = "AAB",
) -> bass.Bass:
    """Build the per-core Bass graph. nw = real words per core."""
    f32 = mybir.dt.float32
    f16 = mybir.dt.float16
    ng = (nw + J - 1) // J  # padded group count
    nwp = ng * J  # padded word count

    nc = bacc.Bacc()
    z_ext = nc.declare_dram_parameter("z", [KP, ng, E], f16, isOutput=False)
    a_ext = nc.declare_dram_parameter("a", [KP, KP], f16, isOutput=False)
    out_ext = nc.declare_dram_parameter("out", [E, nw], f16, isOutput=True)

    engines = {
        "sync": nc.sync,
        "scalar": nc.scalar,
        "gpsimd": nc.gpsimd,
    }

    with ExitStack() as ctx:
        tc = ctx.enter_context(tile.TileContext(nc))
        const = ctx.enter_context(tc.tile_pool(name="const", bufs=1))
        xin = ctx.enter_context(tc.tile_pool(name="xin", bufs=1))
        opool = ctx.enter_context(tc.tile_pool(name="o", bufs=1))
        spool = ctx.enter_context(tc.tile_pool(name="s", bufs=2 * tree_bufs))
        t1pool = ctx.enter_context(tc.tile_pool(name="t1", bufs=tree_bufs))
        u2pool = ctx.enter_context(tc.tile_pool(name="u2", bufs=tree_bufs))
        u3pool = ctx.enter_context(tc.tile_pool(name="u3", bufs=tree_bufs))
        u4pool = ctx.enter_context(tc.tile_pool(name="u4", bufs=tree_bufs))
        pspool = ctx.enter_context(
            tc.tile_pool(name="ps", bufs=psum_bufs, space="PSUM")
        )

        a_t = const.tile([KP, KP], f16)
        nc.sync.dma_start(out=a_t[:, :], in_=a_ext[:, :])
        maxt = opool.tile([E, nwp], f16)

        # Phase A: the entire input stream, issued up front into
        # persistent SBUF tiles, alternating across the HWDGE rings.
        sizes = chunk_plan(ng)
        chunks = []
        g0 = 0
        for ci, gn in enumerate(sizes):
            xt = xin.tile([KP, gn * E], f16, tag=f"x{ci}")
            src = z_ext[:, g0 : g0 + gn, :].rearrange("p g e -> p (g e)")
            engines[rings[ci % len(rings)]].dma_start(out=xt[:, :], in_=src)
            chunks.append((xt, g0, gn))
            g0 += gn

        # Phase B: compute pipeline, cg groups (= 4 PSUM banks) at a time.
        # PSUM columns within a group are w-major (wo*J + j), so the two
        # 20->10 max halves of each group are contiguous 60-element runs.
        HJ = HW * J  # 60
        w_flushed = 0

        def flush_out(upto_words):
            nonlocal w_flushed
            hi = min(upto_words, nw)
            if hi - w_flushed >= flush_words or (hi >= nw and hi > w_flushed):
                engines[out_ring].dma_start(
                    out=out_ext[:, w_flushed:hi], in_=maxt[:, w_flushed:hi]
                )
                w_flushed = hi

        bank_idx = 0
        for xt, gc, gn in chunks:
            # u1 accumulates the 20->10 level for the whole chunk; the
            # rest of the tree runs once per chunk (fewer, bigger DVE ops)
            u1 = t1pool.tile([E, 64 * HJ], f16, tag="u1")
            for s0 in range(0, gn, cg):
                sn = min(cg, gn - s0)
                nb = (sn + 3) // 4
                ps = pspool.tile([E, 4 * BANK], f32, tag="ps")
                for g in range(sn):
                    col = (g // 4) * BANK + (g % 4) * KP
                    nc.tensor.matmul(
                        ps[:, col : col + KP],
                        lhsT=xt[:, (s0 + g) * E : (s0 + g + 1) * E],
                        rhs=a_t[:, :],
                        start=True,
                        stop=True,
                    )
                # Level 1 (20 -> 10) per PSUM bank, path A or B; all APs
                # are 3-dim with >=60-element contiguous runs.
                for b in range(nb):
                    gb = min(4, sn - 4 * b)
                    pl = ps[:, b * BANK : b * BANK + gb * J * W]
                    plo = pl.rearrange("p (g h) -> p g h", h=J * W)[
                        :, :, 0:HJ
                    ]
                    phi = pl.rearrange("p (g h) -> p g h", h=J * W)[
                        :, :, HJ : J * W
                    ]
                    ov = u1[
                        :, (s0 + 4 * b) * HJ : (s0 + 4 * b + gb) * HJ
                    ].rearrange("p (g h) -> p g h", h=HJ)
                    path = bank_pattern[bank_idx % len(bank_pattern)]
                    bank_idx += 1
                    if path == "A":
                        # ACT drains the whole bank; DVE maxes in f16 2x.
                        s = spool.tile([E, 4 * J * W], f16, tag="sa")
                        nc.scalar.copy(s[:, 0 : gb * J * W], pl)
                        sv = s[:, 0 : gb * J * W].rearrange(
                            "p (g h) -> p g h", h=J * W
                        )
                        nc.vector.tensor_max(
                            ov, sv[:, :, 0:HJ], sv[:, :, HJ : J * W]
                        )
                    else:
                        # ACT drains the high halves only; DVE maxes the
                        # PSUM low halves against them (one PSUM input).
                        sh = spool.tile([E, 4 * HJ], f16, tag="sb")
                        shv = sh[:, 0 : gb * HJ].rearrange(
                            "p (g h) -> p g h", h=HJ
                        )
                        nc.scalar.copy(shv, phi)
                        nc.vector.tensor_max(ov, plo, shv)

            # Levels 2-5 for the whole chunk: 10 -> 5 -> (2,2,1) -> 1,
            # all contiguous f16 (runs of 30/12/6 elements).
            def v3(t, width, off, run):
                return t[:, 0 : gn * width].rearrange(
                    "p (g h) -> p g h", h=width
                )[:, :, off : off + run]

            u2 = u2pool.tile([E, 64 * 5 * J], f16, tag="u2")
            nc.vector.tensor_max(
                u2[:, 0 : gn * 5 * J].rearrange("p (g h) -> p g h", h=5 * J),
                v3(u1, HJ, 0, 5 * J),
                v3(u1, HJ, 5 * J, 5 * J),
            )
            u3 = u3pool.tile([E, 64 * 2 * J], f16, tag="u3")
            nc.vector.tensor_max(
                u3[:, 0 : gn * 2 * J].rearrange("p (g h) -> p g h", h=2 * J),
                v3(u2, 5 * J, 0, 2 * J),
                v3(u2, 5 * J, 2 * J, 2 * J),
            )
            u4 = u4pool.tile([E, 64 * J], f16, tag="u4")
            nc.vector.tensor_max(
                u4[:, 0 : gn * J].rearrange("p (g h) -> p g h", h=J),
                v3(u3, 2 * J, 0, J),
                v3(u3, 2 * J, J, J),
            )
            nc.vector.tensor_max(
                maxt[:, gc * J : (gc + gn) * J].rearrange(
                    "p (g h) -> p g h", h=J
                ),
                v3(u4, J, 0, J),
                v3(u2, 5 * J, 4 * J, J),
            )
            flush_out((gc + gn) * J)
    nc.finalize()
    return nc


def kernel(embedded_char, conv_w, conv_b):
    from concourse.bass_utils import run_bass_kernel_spmd

    x = np.asarray(embedded_char, np.float32)
    b_val = float(np.asarray(conv_b, np.float32).reshape(-1)[0])
    B, S, Wl, El = x.shape
    assert (Wl, El) == (W, E)
    bs = B // NCORES
    nw = bs * S
    ng = (nw + J - 1) // J
    a16 = build_conv_matrix(conv_w)

    nc = build_nc(nw)
    in_maps = [
        {
            "z": pack_input(x[i * bs : (i + 1) * bs].reshape(nw, Wl, El), ng),
            "a": a16,
        }
        for i in range(NCORES)
    ]
    res = run_bass_kernel_spmd(nc, in_maps, core_ids=list(range(NCORES)))
    full = np.concatenate(
        [r["out"].astype(np.float32).T.reshape(bs, S, El) for r in res.results], axis=0
    )
    if b_val != 0.0:
        full = full + b_val
    return np.ascontiguousarray(full.astype(np.float32))
